# revision 39
# baseline (speedup 1.0000x reference)
"""Trainium2 Bass kernel for nn_DispersiveLoss (B=2048, D=16*768=12288, 8 cores).

Single-launch design:
  x (2048, 12288) -> 16 row-blocks of 128. Each core holds 7 blocks (a
  "copy" of a fixed 17-edge staircase template H found by search; the 8
  copies jointly cover all 120 block pairs, multiplicities 1-3). Per core,
  7 lhs groups with contiguous rhs windows of widths (5,5,4,4,3,2,1)
  (24 block-columns total incl. the 7 diagonals) accumulate fp8 DoubleRow
  Gram strips over 48 k-steps of K=256.

  sq (row norms) are extracted on-device from the Gram diagonals
  (identity-mask reduce), transposed via PE, and folded into PSUM with
  bf16 K=1 rank-1 matmuls so PSUM holds p = g - (sq_r + sq_c)/2 = -d2/2.

  Stats use a host-built elementwise weight matrix wmat (1/multiplicity
  per pair block, triangle-mask/diag-mult on diag blocks, 0 on pads):
    E  = sum wmat*exp(2*SS*p),  S1 = sum wmat*p,  S2 = sum wmat*p^2,
    feat = sum wd*sqrt(sq).
  Host combines per-core [128,4] partials in float64.
"""

import os

import numpy as np
import ml_dtypes

import concourse.bass as bass
import concourse.bass_utils as _bu
import concourse.mybir as mybir
import concourse.tile as tile
from concourse import bacc
from concourse.bass_utils import run_bass_kernel_spmd

# (walrus's --enable-ldw-opt=true was tried here: it crashes codegen on
# TRN2 InstLdweights, which is presumably why concourse pins it off)

NC_N = 8
B, D = 2048, 12288
BLK = 128
KCH = 96          # k-chunks of 128
NSTEP = 48        # DoubleRow k-steps of 256
KB = 8            # k-chunks per DMA batch
NBATCH = KCH // KB
TAU = 0.5
SS = 1.0 / (D * TAU)
G = 7             # blocks per core
SLABW = G * BLK   # 896

F32 = mybir.dt.float32
BF16 = mybir.dt.bfloat16
FP16 = mybir.dt.float16
FP8 = mybir.dt.float8e4
NP_FP8 = ml_dtypes.float8_e4m3
NP_BF16 = ml_dtypes.bfloat16

# ---- covering design (found by search; see transcript) ----
W_WIDTHS = (5, 5, 4, 4, 3, 2, 1)
COPIES = [
    [10, 2, 3, 0, 12, 11, 8],
    [14, 8, 11, 10, 1, 9, 15],
    [6, 2, 8, 13, 0, 5, 15],
    [2, 9, 1, 7, 4, 0, 14],
    [8, 7, 15, 4, 3, 11, 6],
    [11, 1, 9, 13, 5, 3, 14],
    [12, 4, 13, 5, 7, 10, 6],
    [2, 15, 1, 6, 14, 12, 9],
]

# PSUM layout: per group (base, [piece col ranges]) bank-padded (512 f32 banks)
GROUP_BASE = (0, 1024, 2048, 2560, 3072, 3584, 3840)
GROUP_PIECES = (
    ((0, 512), (512, 640)),
    ((1024, 1536), (1536, 1664)),
    ((2048, 2560),),
    ((2560, 3072),),
    ((3072, 3456),),
    ((3584, 3840),),
    ((3840, 3968),),
)
PSUM_W = 3968
# transpose scratch inside g0's pad (bank 1, cols 640:768)
PT_OFF = 640
# stats ranges (contiguous real columns)
RANGES = ((0, 640), (1024, 1664), (2048, 3072), (3072, 3456), (3584, 3968))
# engine split: index into RANGES handled by DVE vs GpSimd
DVE_RANGES = (2, 3)
GPS_RANGES = (0, 1, 4)

N_PAIRS = B * (B - 1) // 2

KERNEL_EXEC_NS = []

_cache = {}


def _trace_enabled():
    return bool(os.environ.get("KERNEL_TRACE"))


def _multiplicities():
    from collections import defaultdict

    H = []
    for i, wi in enumerate(W_WIDTHS):
        for j in range(i + 1, i + wi):
            H.append((i, j))
    mult = defaultdict(int)
    dmult = defaultdict(int)
    for phi in COPIES:
        for (i, j) in H:
            a, b = phi[i], phi[j]
            mult[(min(a, b), max(a, b))] += 1
        for b in phi:
            dmult[b] += 1
    return mult, dmult


MULT, DMULT = _multiplicities()


def _build_kernel():
    nc = bacc.Bacc("TRN2", target_bir_lowering=False, debug=False, num_devices=NC_N)
    xT = nc.dram_tensor("xT", [BLK, KCH, SLABW], FP8, kind="ExternalInput")
    wmat_d = nc.dram_tensor("wmat", [BLK, PSUM_W], F32, kind="ExternalInput")
    wmat16_d = nc.dram_tensor("wmat16", [BLK, PSUM_W], FP16, kind="ExternalInput")
    idm_d = nc.dram_tensor("idm", [BLK, BLK], F32, kind="ExternalInput")
    wd_d = nc.dram_tensor("wd", [BLK, 8], F32, kind="ExternalInput")
    out_stats = nc.dram_tensor("out_stats", [BLK, 4], F32, kind="ExternalOutput")

    MULT_OP = mybir.AluOpType.mult
    ADD_OP = mybir.AluOpType.add
    EXP = mybir.ActivationFunctionType.Exp
    SQRT = mybir.ActivationFunctionType.Sqrt
    DR = mybir.MatmulPerfMode.DoubleRow

    with tile.TileContext(nc) as tc:
        with (
            tc.tile_pool(name="slab", bufs=4) as slab_pool,
            tc.tile_pool(name="psp", bufs=1, space="PSUM") as psp,
            tc.tile_pool(name="aux", bufs=1) as aux,
            tc.tile_pool(name="scr", bufs=1) as scr_pool,
            tc.tile_pool(name="accp", bufs=1) as accp,
            tc.tile_pool(name="drp", bufs=1, space="DRAM") as dram_pool,
        ):
            P = psp.tile([BLK, 4096], F32)
            acc = accp.tile([BLK, 16], F32)
            sqcol = accp.tile([BLK, 8], F32)

            # ---- aux inputs + constants ----
            idm = aux.tile([BLK, BLK], F32)
            wdm = aux.tile([BLK, 8], F32)
            onesbf = aux.tile([1, SLABW], BF16)
            nc.gpsimd.memset(onesbf[:], 1.0)

            # PE warm-up + Exp/Sqrt act-table preload while first DMA lands
            warm = aux.tile([BLK, BLK], FP8)
            nc.gpsimd.memset(warm[:], 0.0)
            for _ in range(20):
                nc.tensor.matmul(
                    P[:, 3968:4096], warm[:], warm[:], start=True, stop=True,
                    skip_group_check=True,
                )
            # pre-zero co-resident regions (start=True resets the whole PSUM
            # bank, so groups sharing a bank must accumulate with start=False
            # onto a zeroed region): g6 (bank 7) and the transpose pad (bank 1)
            nc.tensor.matmul(P[:, 3840:3968], warm[:], warm[:], start=True,
                             stop=True, skip_group_check=True)
            nc.tensor.matmul(P[:, 640:768], warm[:], warm[:], start=True,
                             stop=True, skip_group_check=True)
            zc = aux.tile([BLK, 1], F32)
            nc.gpsimd.memset(zc[:], 0.0)
            zs = aux.tile([BLK, 1], F32)
            nc.scalar.activation(zs[:], zc[:], SQRT)
            zs2 = aux.tile([BLK, 1], F32)
            nc.scalar.activation(zs2[:], zc[:], EXP)

            # ---- main stream ----
            # kstep 0: bank-aligned pieces with start=True (PSUM bank reset);
            # ksteps 1..47: full-window matmuls (start=False accumulate may
            # cross banks), 7 matmuls per kstep keeps LDWEIGHTS off the
            # critical path. wmat arrives mid-stream (needed only at tail).
            wmat = aux.tile([BLK, PSUM_W], F32)
            wmat16 = aux.tile([BLK, PSUM_W], FP16)
            # first batch split small so the PE can start sooner
            batches = [(0, 2), (2, 6)] + [(k, KB) for k in range(KB, KCH, KB)]
            for bi, (kc0, klen) in enumerate(batches):
                st = slab_pool.tile([BLK, klen, SLABW], FP8, tag=f"slab{min(bi,2)}")
                nc.sync.dma_start(st[:], xT[:, kc0 : kc0 + klen, :])
                if bi == 1:
                    nc.sync.dma_start(idm[:], idm_d[:])
                    nc.sync.dma_start(wdm[:], wd_d[:])
                if bi == 4:
                    nc.sync.dma_start(wmat[:], wmat_d[:])
                    nc.sync.dma_start(wmat16[:], wmat16_d[:])
                # bank-aligned pieces (matmul dest must stay in one bank),
                # ordered big->small to amortize LDWEIGHTS
                order = [(0, 0), (1, 0), (2, 0), (3, 0), (4, 0),
                         (5, 0), (0, 1), (1, 1), (6, 0)]
                last_batch = bi == len(batches) - 1
                if last_batch:
                    # piece-major: each diag-holding piece finishes its whole
                    # k-sweep early so sq extraction overlaps the remaining
                    # stream instead of trailing it
                    for (g, pi) in order:
                        pc0, pc1 = GROUP_PIECES[g][pi]
                        base = GROUP_BASE[g]
                        w0 = g * BLK + (pc0 - base)
                        for ii in range(0, klen, 2):
                            lhs = st[:, ii : ii + 2, g * BLK : (g + 1) * BLK]
                            rhs = st[:, ii : ii + 2, w0 : w0 + (pc1 - pc0)]
                            nc.tensor.matmul(
                                P[:, pc0:pc1], lhs, rhs,
                                start=False, stop=(ii == klen - 2),
                                perf_mode=DR, skip_group_check=(g == 6),
                            )
                else:
                    for ii in range(0, klen, 2):
                        ks = kc0 + ii  # k-chunk index
                        first = ks == 0
                        for (g, pi) in order:
                            pc0, pc1 = GROUP_PIECES[g][pi]
                            base = GROUP_BASE[g]
                            w0 = g * BLK + (pc0 - base)
                            lhs = st[:, ii : ii + 2, g * BLK : (g + 1) * BLK]
                            rhs = st[:, ii : ii + 2, w0 : w0 + (pc1 - pc0)]
                            # g6 shares bank 7 with g5: g5's kstep-0 start
                            # resets the bank; g6 accumulates onto zeros
                            nc.tensor.matmul(
                                P[:, pc0:pc1], lhs, rhs,
                                start=(first and g != 6), stop=False,
                                perf_mode=DR, skip_group_check=(g == 6),
                            )

            # ---- sq extraction from Gram diagonals ----
            # keep-alive zero-accumulate matmuls hold the PE p-state up while
            # DVE extracts and the sq roundtrip completes (adds 0, bank-safe)
            for g in range(G):
                base = GROUP_BASE[g]
                dscr = scr_pool.tile([BLK, BLK], F32, tag=f"dscr{g}")
                nc.vector.scalar_tensor_tensor(
                    out=dscr[:], in0=P[:, base : base + BLK], scalar=1.0,
                    in1=idm[:], op0=MULT_OP, op1=MULT_OP,
                    accum_out=sqcol[:, g : g + 1],
                )
                nc.tensor.matmul(
                    P[:, 3968:4096], warm[:], warm[:], start=False, stop=False,
                    skip_group_check=True,
                )
            nc.gpsimd.memset(sqcol[:, 7:8], 0.0)

            # ---- transpose sq to single-partition layout (via PE + DRAM) ----
            pt = P[0:8, PT_OFF : PT_OFF + BLK]
            nc.tensor.matmul(pt, sqcol[:], idm[:], is_transpose=True,
                             start=False, stop=True, skip_group_check=True)
            sqTbf = aux.tile([8, BLK], BF16)
            nc.vector.tensor_scalar(
                out=sqTbf[:], in0=pt, scalar1=-0.5, scalar2=0.0,
                op0=MULT_OP, op1=ADD_OP,
            )
            sqcat = aux.tile([1, 8 * BLK], BF16)
            sq_rt = dram_pool.tile([8, BLK], BF16)
            nc.sync.dma_start(sq_rt[:], sqTbf[:])
            nc.sync.dma_start(
                sqcat[:],
                sq_rt[:].rearrange("p f -> (p f)").rearrange("(a f) -> a f", a=1),
            )
            for _ in range(16):
                nc.tensor.matmul(
                    P[:, 3968:4096], warm[:], warm[:], start=False, stop=False,
                    skip_group_check=True,
                )

            # ---- rank-1 folds: p := g - sq_r/2 - sq_c/2 (per bank piece) ----
            for g in range(G):
                base = GROUP_BASE[g]
                npieces = len(GROUP_PIECES[g])
                for pi, (pc0, pc1) in enumerate(GROUP_PIECES[g]):
                    lastp = pi == npieces - 1
                    wlen = pc1 - pc0
                    rstart = g * BLK + (pc0 - base)
                    # per-row: adds -sq_lhs[m]/2 to every column
                    nc.tensor.matmul(
                        P[:, pc0:pc1],
                        sqcat[:, g * BLK : (g + 1) * BLK],
                        onesbf[:, 0:wlen],
                        start=False, stop=False, skip_group_check=True,
                    )
                    # per-col: adds -sq_rhs[n]/2 to every row
                    nc.tensor.matmul(
                        P[:, pc0:pc1],
                        onesbf[:, 0:BLK],
                        sqcat[:, rstart : rstart + wlen],
                        start=False, stop=lastp, skip_group_check=True,
                    )

            # ---- stats passes ----
            # acc cols: 0:5 Ew per range, 5:10 S1, 10:15 S2, 15 feat
            # Ew runs on fp16 (DVE 2x); S1/S2 stay f32 from PSUM (fp16/bf16
            # staging of p measurably inflates the pair-std variance).
            for ri, (c0, c1) in enumerate(RANGES):
                n = c1 - c0
                # E = sum wmat*exp(2*SS*p): ACT exp fp16, DVE 2x weighting
                et = scr_pool.tile([BLK, n], FP16, tag=f"et{ri}")
                nc.scalar.activation(et[:], P[:, c0:c1], EXP, scale=2.0 * SS)
                ew = scr_pool.tile([BLK, n], FP16, tag=f"ew{ri}")
                nc.vector.scalar_tensor_tensor(
                    out=ew[:], in0=et[:], scalar=1.0, in1=wmat16[:, c0:c1],
                    op0=MULT_OP, op1=MULT_OP, accum_out=acc[:, ri : 1 + ri],
                )
                # S1 = sum wmat*p  (f32, out wp reused for S2)
                wp = scr_pool.tile([BLK, n], F32, tag=f"wp{ri}")
                nc.vector.scalar_tensor_tensor(
                    out=wp[:], in0=P[:, c0:c1], scalar=1.0, in1=wmat[:, c0:c1],
                    op0=MULT_OP, op1=MULT_OP, accum_out=acc[:, 5 + ri : 6 + ri],
                )
                # S2 = sum wp*p (f32)
                s2s = scr_pool.tile([BLK, n], F32, tag=f"s2{ri}")
                nc.vector.scalar_tensor_tensor(
                    out=s2s[:], in0=wp[:], scalar=1.0, in1=P[:, c0:c1],
                    op0=MULT_OP, op1=MULT_OP, accum_out=acc[:, 10 + ri : 11 + ri],
                )

            # feat partial: sum wd * sqrt(sq)  (single act-table swap, last)
            srt = aux.tile([BLK, 8], F32)
            nc.scalar.activation(srt[:], sqcol[:], SQRT)
            fscr = aux.tile([BLK, 8], F32)
            nc.vector.scalar_tensor_tensor(
                out=fscr[:], in0=srt[:], scalar=1.0, in1=wdm[:],
                op0=MULT_OP, op1=MULT_OP, accum_out=acc[:, 15:16],
            )

            outt = accp.tile([BLK, 4], F32)
            nc.vector.tensor_reduce(outt[:, 0:1], acc[:, 0:5], mybir.AxisListType.X, ADD_OP)
            nc.vector.tensor_reduce(outt[:, 1:2], acc[:, 5:10], mybir.AxisListType.X, ADD_OP)
            nc.vector.tensor_reduce(outt[:, 2:3], acc[:, 10:15], mybir.AxisListType.X, ADD_OP)
            nc.vector.tensor_copy(outt[:, 3:4], acc[:, 15:16])
            nc.sync.dma_start(out_stats[:], outt[:])
    nc.compile()
    return nc


def _get(name, builder):
    if name not in _cache:
        _cache[name] = builder()
    return _cache[name]


def _run(nc, in_maps, tag):
    if _trace_enabled():
        try:
            import profhook

            profhook.install()
        except Exception:
            pass
        import tempfile

        res = run_bass_kernel_spmd(
            nc, in_maps, list(range(NC_N)), trace=True,
            tmpdir=tempfile.mkdtemp(prefix=f"ktrace_{tag}_"),
        )
        KERNEL_EXEC_NS.append((tag, res.exec_time_ns))
        return res.results
    return run_bass_kernel_spmd(nc, in_maps, list(range(NC_N))).results


def _host_inputs(x):
    xq = x.astype(NP_FP8)
    xT = np.ascontiguousarray(xq.T)  # (D, B)
    xTv = xT.reshape(KCH, BLK, B)
    tri = np.triu(np.ones((BLK, BLK), np.float32), k=1)
    idm = np.eye(BLK, dtype=np.float32)
    maps = []
    for c in range(NC_N):
        blocks = COPIES[c]
        slab = np.concatenate(
            [xTv[:, :, b * BLK : (b + 1) * BLK] for b in blocks], axis=2
        ).transpose(1, 0, 2)  # [128, 96, 896]
        wmat = np.zeros((BLK, PSUM_W), np.float32)
        for g in range(G):
            base = GROUP_BASE[g]
            for r in range(g, g + W_WIDTHS[g]):
                col = base + (r - g) * BLK
                if r == g:
                    wmat[:, col : col + BLK] = tri / DMULT[blocks[g]]
                else:
                    a, b = blocks[g], blocks[r]
                    wmat[:, col : col + BLK] = 1.0 / MULT[(min(a, b), max(a, b))]
        wd = np.zeros((BLK, 8), np.float32)
        for g in range(G):
            wd[:, g] = 1.0 / DMULT[blocks[g]]
        maps.append(
            {
                "xT": np.ascontiguousarray(slab),
                "wmat": wmat,
                "wmat16": wmat.astype(np.float16),
                "idm": idm,
                "wd": wd,
            }
        )
    return maps


def kernel(features):
    x = np.asarray(features).reshape(B, D)
    maps = _host_inputs(x)
    nc = _get("main", _build_kernel)
    res = _run(nc, maps, "main")

    E = S1 = S2 = FEAT = 0.0
    for c in range(NC_N):
        o = res[c]["out_stats"].astype(np.float64)
        E += o[:, 0].sum()
        S1 += o[:, 1].sum()
        S2 += o[:, 2].sum()
        FEAT += o[:, 3].sum()

    N = float(N_PAIRS)
    mean_d2 = -2.0 * S1 / N
    mean = mean_d2 / D
    var_d2 = (4.0 * S2 - N * mean_d2 * mean_d2) / (N - 1.0)
    std = np.sqrt(var_d2) / D
    loss = -np.log(E) + np.log(N)
    feat = FEAT / B

    return (
        np.float32(loss),
        np.float32(feat),
        np.float32(mean),
        np.float32(std),
    )


if __name__ == "__main__":
    f = np.random.default_rng(0).standard_normal((B, 16, 768), dtype=np.float32)
    print(kernel(features=f))


# revision 40
# speedup vs baseline: 1.1755x; 1.1755x over previous
"""Trainium2 Bass kernel for nn_DispersiveLoss (B=2048, D=16*768=12288, 8 cores).

Single-launch design:
  x (2048, 12288) -> 16 row-blocks of 128. Each core holds 7 blocks (a
  "copy" of a fixed 17-edge staircase template H found by search; the 8
  copies jointly cover all 120 block pairs, multiplicities 1-3). Per core,
  7 lhs groups with contiguous rhs windows of widths (5,5,4,4,3,2,1)
  (24 block-columns total incl. the 7 diagonals) accumulate fp8 DoubleRow
  Gram strips over 48 k-steps of K=256.

  sq (row norms) are extracted on-device from the Gram diagonals
  (identity-mask reduce), transposed via PE, and folded into PSUM with
  bf16 K=1 rank-1 matmuls so PSUM holds p = g - (sq_r + sq_c)/2 = -d2/2.

  Stats use a host-built elementwise weight matrix wmat (1/multiplicity
  per pair block, triangle-mask/diag-mult on diag blocks, 0 on pads):
    E  = sum wmat*exp(2*SS*p),  S1 = sum wmat*p,  S2 = sum wmat*p^2,
    feat = sum wd*sqrt(sq).
  Host combines per-core [128,4] partials in float64.
"""

import os

import numpy as np
import ml_dtypes

import concourse.bass as bass
import concourse.bass_utils as _bu
import concourse.mybir as mybir
import concourse.tile as tile
from concourse import bacc
from concourse.bass_utils import run_bass_kernel_spmd

# (walrus's --enable-ldw-opt=true was tried here: it crashes codegen on
# TRN2 InstLdweights, which is presumably why concourse pins it off)

NC_N = 8
B, D = 2048, 12288
BLK = 128
KCH = 96          # k-chunks of 128
NSTEP = 48        # DoubleRow k-steps of 256
KB = 8            # k-chunks per DMA batch
NBATCH = KCH // KB
TAU = 0.5
SS = 1.0 / (D * TAU)
G = 7             # blocks per core
SLABW = G * BLK   # 896

F32 = mybir.dt.float32
BF16 = mybir.dt.bfloat16
FP16 = mybir.dt.float16
FP8 = mybir.dt.float8e4
NP_FP8 = ml_dtypes.float8_e4m3
NP_BF16 = ml_dtypes.bfloat16

# ---- covering design (found by search; see transcript) ----
W_WIDTHS = (5, 5, 4, 4, 3, 2, 1)
COPIES = [
    [10, 2, 3, 0, 12, 11, 8],
    [14, 8, 11, 10, 1, 9, 15],
    [6, 2, 8, 13, 0, 5, 15],
    [2, 9, 1, 7, 4, 0, 14],
    [8, 7, 15, 4, 3, 11, 6],
    [11, 1, 9, 13, 5, 3, 14],
    [12, 4, 13, 5, 7, 10, 6],
    [2, 15, 1, 6, 14, 12, 9],
]

# PSUM layout: per group (base, [piece col ranges]) bank-padded (512 f32 banks)
GROUP_BASE = (0, 1024, 2048, 2560, 3072, 3584, 3840)
GROUP_PIECES = (
    ((0, 512), (512, 640)),
    ((1024, 1536), (1536, 1664)),
    ((2048, 2560),),
    ((2560, 3072),),
    ((3072, 3456),),
    ((3584, 3840),),
    ((3840, 3968),),
)
PSUM_W = 3968
# transpose scratch inside g0's pad (bank 1, cols 640:768)
PT_OFF = 640
# stats ranges (contiguous real columns)
RANGES = ((0, 640), (1024, 1664), (2048, 3072), (3072, 3456), (3584, 3968))
# engine split: index into RANGES handled by DVE vs GpSimd
DVE_RANGES = (2, 3)
GPS_RANGES = (0, 1, 4)

N_PAIRS = B * (B - 1) // 2

KERNEL_EXEC_NS = []

_cache = {}


def _trace_enabled():
    return bool(os.environ.get("KERNEL_TRACE"))


def _multiplicities():
    from collections import defaultdict

    H = []
    for i, wi in enumerate(W_WIDTHS):
        for j in range(i + 1, i + wi):
            H.append((i, j))
    mult = defaultdict(int)
    dmult = defaultdict(int)
    for phi in COPIES:
        for (i, j) in H:
            a, b = phi[i], phi[j]
            mult[(min(a, b), max(a, b))] += 1
        for b in phi:
            dmult[b] += 1
    return mult, dmult


MULT, DMULT = _multiplicities()


def _build_kernel():
    nc = bacc.Bacc("TRN2", target_bir_lowering=False, debug=False, num_devices=NC_N)
    xT = nc.dram_tensor("xT", [BLK, KCH, SLABW], FP8, kind="ExternalInput")
    wmat_d = nc.dram_tensor("wmat", [BLK, PSUM_W], F32, kind="ExternalInput")
    wmat16_d = nc.dram_tensor("wmat16", [BLK, PSUM_W], FP16, kind="ExternalInput")
    idm_d = nc.dram_tensor("idm", [BLK, BLK], F32, kind="ExternalInput")
    wd_d = nc.dram_tensor("wd", [BLK, 8], F32, kind="ExternalInput")
    out_stats = nc.dram_tensor("out_stats", [BLK, 4], F32, kind="ExternalOutput")

    MULT_OP = mybir.AluOpType.mult
    ADD_OP = mybir.AluOpType.add
    EXP = mybir.ActivationFunctionType.Exp
    SQRT = mybir.ActivationFunctionType.Sqrt
    DR = mybir.MatmulPerfMode.DoubleRow

    with tile.TileContext(nc) as tc:
        with (
            tc.tile_pool(name="slab", bufs=4) as slab_pool,
            tc.tile_pool(name="psp", bufs=1, space="PSUM") as psp,
            tc.tile_pool(name="aux", bufs=1) as aux,
            tc.tile_pool(name="scr", bufs=1) as scr_pool,
            tc.tile_pool(name="accp", bufs=1) as accp,
            tc.tile_pool(name="drp", bufs=1, space="DRAM") as dram_pool,
        ):
            P = psp.tile([BLK, 4096], F32)
            acc = accp.tile([BLK, 16], F32)
            sqcol = accp.tile([BLK, 8], F32)

            # ---- aux inputs + constants ----
            idm = aux.tile([BLK, BLK], F32)
            wdm = aux.tile([BLK, 8], F32)
            onesbf = aux.tile([1, SLABW], BF16)
            nc.gpsimd.memset(onesbf[:], 1.0)

            # PE warm-up + Exp/Sqrt act-table preload while first DMA lands
            warm = aux.tile([BLK, BLK], FP8)
            nc.gpsimd.memset(warm[:], 0.0)
            for _ in range(20):
                nc.tensor.matmul(
                    P[:, 3968:4096], warm[:], warm[:], start=True, stop=True,
                    skip_group_check=True,
                )
            # pre-zero co-resident regions (start=True resets the whole PSUM
            # bank, so groups sharing a bank must accumulate with start=False
            # onto a zeroed region): g6 (bank 7) and the transpose pad (bank 1)
            nc.tensor.matmul(P[:, 3840:3968], warm[:], warm[:], start=True,
                             stop=True, skip_group_check=True)
            nc.tensor.matmul(P[:, 640:768], warm[:], warm[:], start=True,
                             stop=True, skip_group_check=True)
            zc = aux.tile([BLK, 1], F32)
            nc.gpsimd.memset(zc[:], 0.0)
            zs = aux.tile([BLK, 1], F32)
            nc.scalar.activation(zs[:], zc[:], SQRT)
            zs2 = aux.tile([BLK, 1], F32)
            nc.scalar.activation(zs2[:], zc[:], EXP)

            # ---- main stream ----
            # kstep 0: bank-aligned pieces with start=True (PSUM bank reset);
            # ksteps 1..47: full-window matmuls (start=False accumulate may
            # cross banks), 7 matmuls per kstep keeps LDWEIGHTS off the
            # critical path. wmat arrives mid-stream (needed only at tail).
            wmat = aux.tile([BLK, PSUM_W], F32)
            wmat16 = aux.tile([BLK, PSUM_W], FP16)
            # first batch split small so the PE can start sooner
            batches = [(0, 2), (2, 6)] + [(k, KB) for k in range(KB, KCH, KB)]
            for bi, (kc0, klen) in enumerate(batches):
                st = slab_pool.tile([BLK, klen, SLABW], FP8, tag=f"slab{min(bi,2)}")
                nc.sync.dma_start(st[:], xT[:, kc0 : kc0 + klen, :])
                if bi == 1:
                    nc.sync.dma_start(idm[:], idm_d[:])
                    nc.sync.dma_start(wdm[:], wd_d[:])
                if bi == 4:
                    nc.sync.dma_start(wmat[:], wmat_d[:])
                    nc.sync.dma_start(wmat16[:], wmat16_d[:])
                for ii in range(0, klen, 2):
                    ks = kc0 + ii  # k-chunk index
                    first = ks == 0
                    last = ks == KCH - 2
                    # bank-aligned pieces (matmul dest must stay in one bank),
                    # ordered big->small to amortize LDWEIGHTS
                    order = [(0, 0), (1, 0), (2, 0), (3, 0), (4, 0),
                             (5, 0), (0, 1), (1, 1), (6, 0)]
                    for (g, pi) in order:
                        pc0, pc1 = GROUP_PIECES[g][pi]
                        base = GROUP_BASE[g]
                        w0 = g * BLK + (pc0 - base)
                        lhs = st[:, ii : ii + 2, g * BLK : (g + 1) * BLK]
                        rhs = st[:, ii : ii + 2, w0 : w0 + (pc1 - pc0)]
                        # g6 shares bank 7 with g5: g5's kstep-0 start resets
                        # the bank; g6 accumulates onto the zeroed region
                        nc.tensor.matmul(
                            P[:, pc0:pc1], lhs, rhs,
                            start=(first and g != 6), stop=last, perf_mode=DR,
                            skip_group_check=(g == 6),
                        )

            # ---- sq extraction from Gram diagonals ----
            # keep-alive zero-accumulate matmuls hold the PE p-state up while
            # DVE extracts and the sq roundtrip completes (adds 0, bank-safe)
            for g in range(G):
                base = GROUP_BASE[g]
                dscr = scr_pool.tile([BLK, BLK], F32, tag=f"dscr{g}")
                nc.vector.scalar_tensor_tensor(
                    out=dscr[:], in0=P[:, base : base + BLK], scalar=1.0,
                    in1=idm[:], op0=MULT_OP, op1=MULT_OP,
                    accum_out=sqcol[:, g : g + 1],
                )
                nc.tensor.matmul(
                    P[:, 3968:4096], warm[:], warm[:], start=False, stop=False,
                    skip_group_check=True,
                )
            nc.gpsimd.memset(sqcol[:, 7:8], 0.0)

            # ---- transpose sq to single-partition layout (via PE + DRAM) ----
            pt = P[0:8, PT_OFF : PT_OFF + BLK]
            nc.tensor.matmul(pt, sqcol[:], idm[:], is_transpose=True,
                             start=False, stop=True, skip_group_check=True)
            sqTbf = aux.tile([8, BLK], BF16)
            nc.vector.tensor_scalar(
                out=sqTbf[:], in0=pt, scalar1=-0.5, scalar2=0.0,
                op0=MULT_OP, op1=ADD_OP,
            )
            sqcat = aux.tile([1, 8 * BLK], BF16)
            sq_rt = dram_pool.tile([8, BLK], BF16)
            nc.sync.dma_start(sq_rt[:], sqTbf[:])
            nc.sync.dma_start(
                sqcat[:],
                sq_rt[:].rearrange("p f -> (p f)").rearrange("(a f) -> a f", a=1),
            )
            for _ in range(16):
                nc.tensor.matmul(
                    P[:, 3968:4096], warm[:], warm[:], start=False, stop=False,
                    skip_group_check=True,
                )

            # ---- rank-1 folds: p := g - sq_r/2 - sq_c/2 (per bank piece) ----
            for g in range(G):
                base = GROUP_BASE[g]
                npieces = len(GROUP_PIECES[g])
                for pi, (pc0, pc1) in enumerate(GROUP_PIECES[g]):
                    lastp = pi == npieces - 1
                    wlen = pc1 - pc0
                    rstart = g * BLK + (pc0 - base)
                    # per-row: adds -sq_lhs[m]/2 to every column
                    nc.tensor.matmul(
                        P[:, pc0:pc1],
                        sqcat[:, g * BLK : (g + 1) * BLK],
                        onesbf[:, 0:wlen],
                        start=False, stop=False, skip_group_check=True,
                    )
                    # per-col: adds -sq_rhs[n]/2 to every row
                    nc.tensor.matmul(
                        P[:, pc0:pc1],
                        onesbf[:, 0:BLK],
                        sqcat[:, rstart : rstart + wlen],
                        start=False, stop=lastp, skip_group_check=True,
                    )

            # ---- stats passes ----
            # acc cols: 0:5 Ew per range, 5:10 S1, 10:15 S2, 15 feat
            # Ew runs on fp16 (DVE 2x); S1/S2 stay f32 from PSUM (fp16/bf16
            # staging of p measurably inflates the pair-std variance).
            for ri, (c0, c1) in enumerate(RANGES):
                n = c1 - c0
                # E = sum wmat*exp(2*SS*p): ACT exp fp16, DVE 2x weighting
                et = scr_pool.tile([BLK, n], FP16, tag=f"et{ri}")
                nc.scalar.activation(et[:], P[:, c0:c1], EXP, scale=2.0 * SS)
                ew = scr_pool.tile([BLK, n], FP16, tag=f"ew{ri}")
                nc.vector.scalar_tensor_tensor(
                    out=ew[:], in0=et[:], scalar=1.0, in1=wmat16[:, c0:c1],
                    op0=MULT_OP, op1=MULT_OP, accum_out=acc[:, ri : 1 + ri],
                )
                # S1 = sum wmat*p  (f32, out wp reused for S2)
                wp = scr_pool.tile([BLK, n], F32, tag=f"wp{ri}")
                nc.vector.scalar_tensor_tensor(
                    out=wp[:], in0=P[:, c0:c1], scalar=1.0, in1=wmat[:, c0:c1],
                    op0=MULT_OP, op1=MULT_OP, accum_out=acc[:, 5 + ri : 6 + ri],
                )
                # S2 = sum wp*p (f32)
                s2s = scr_pool.tile([BLK, n], F32, tag=f"s2{ri}")
                nc.vector.scalar_tensor_tensor(
                    out=s2s[:], in0=wp[:], scalar=1.0, in1=P[:, c0:c1],
                    op0=MULT_OP, op1=MULT_OP, accum_out=acc[:, 10 + ri : 11 + ri],
                )

            # feat partial: sum wd * sqrt(sq)  (single act-table swap, last)
            srt = aux.tile([BLK, 8], F32)
            nc.scalar.activation(srt[:], sqcol[:], SQRT)
            fscr = aux.tile([BLK, 8], F32)
            nc.vector.scalar_tensor_tensor(
                out=fscr[:], in0=srt[:], scalar=1.0, in1=wdm[:],
                op0=MULT_OP, op1=MULT_OP, accum_out=acc[:, 15:16],
            )

            outt = accp.tile([BLK, 4], F32)
            nc.vector.tensor_reduce(outt[:, 0:1], acc[:, 0:5], mybir.AxisListType.X, ADD_OP)
            nc.vector.tensor_reduce(outt[:, 1:2], acc[:, 5:10], mybir.AxisListType.X, ADD_OP)
            nc.vector.tensor_reduce(outt[:, 2:3], acc[:, 10:15], mybir.AxisListType.X, ADD_OP)
            nc.vector.tensor_copy(outt[:, 3:4], acc[:, 15:16])
            nc.sync.dma_start(out_stats[:], outt[:])
    nc.compile()
    return nc


def _get(name, builder):
    if name not in _cache:
        _cache[name] = builder()
    return _cache[name]


def _run(nc, in_maps, tag):
    if _trace_enabled():
        try:
            import profhook

            profhook.install()
        except Exception:
            pass
        import tempfile

        res = run_bass_kernel_spmd(
            nc, in_maps, list(range(NC_N)), trace=True,
            tmpdir=tempfile.mkdtemp(prefix=f"ktrace_{tag}_"),
        )
        KERNEL_EXEC_NS.append((tag, res.exec_time_ns))
        return res.results
    return run_bass_kernel_spmd(nc, in_maps, list(range(NC_N))).results


def _host_inputs(x):
    xq = x.astype(NP_FP8)
    xT = np.ascontiguousarray(xq.T)  # (D, B)
    xTv = xT.reshape(KCH, BLK, B)
    tri = np.triu(np.ones((BLK, BLK), np.float32), k=1)
    idm = np.eye(BLK, dtype=np.float32)
    maps = []
    for c in range(NC_N):
        blocks = COPIES[c]
        slab = np.concatenate(
            [xTv[:, :, b * BLK : (b + 1) * BLK] for b in blocks], axis=2
        ).transpose(1, 0, 2)  # [128, 96, 896]
        wmat = np.zeros((BLK, PSUM_W), np.float32)
        for g in range(G):
            base = GROUP_BASE[g]
            for r in range(g, g + W_WIDTHS[g]):
                col = base + (r - g) * BLK
                if r == g:
                    wmat[:, col : col + BLK] = tri / DMULT[blocks[g]]
                else:
                    a, b = blocks[g], blocks[r]
                    wmat[:, col : col + BLK] = 1.0 / MULT[(min(a, b), max(a, b))]
        wd = np.zeros((BLK, 8), np.float32)
        for g in range(G):
            wd[:, g] = 1.0 / DMULT[blocks[g]]
        maps.append(
            {
                "xT": np.ascontiguousarray(slab),
                "wmat": wmat,
                "wmat16": wmat.astype(np.float16),
                "idm": idm,
                "wd": wd,
            }
        )
    return maps


def kernel(features):
    x = np.asarray(features).reshape(B, D)
    maps = _host_inputs(x)
    nc = _get("main", _build_kernel)
    res = _run(nc, maps, "main")

    E = S1 = S2 = FEAT = 0.0
    for c in range(NC_N):
        o = res[c]["out_stats"].astype(np.float64)
        E += o[:, 0].sum()
        S1 += o[:, 1].sum()
        S2 += o[:, 2].sum()
        FEAT += o[:, 3].sum()

    N = float(N_PAIRS)
    mean_d2 = -2.0 * S1 / N
    mean = mean_d2 / D
    var_d2 = (4.0 * S2 - N * mean_d2 * mean_d2) / (N - 1.0)
    std = np.sqrt(var_d2) / D
    loss = -np.log(E) + np.log(N)
    feat = FEAT / B

    return (
        np.float32(loss),
        np.float32(feat),
        np.float32(mean),
        np.float32(std),
    )


if __name__ == "__main__":
    f = np.random.default_rng(0).standard_normal((B, 16, 768), dtype=np.float32)
    print(kernel(features=f))


# revision 47
# speedup vs baseline: 1.1820x; 1.0055x over previous
"""Trainium2 Bass kernel for nn_DispersiveLoss (B=2048, D=16*768=12288, 8 cores).

Single-launch design:
  x (2048, 12288) -> 16 row-blocks of 128. Each core holds 7 blocks (a
  "copy" of a fixed 17-edge staircase template H found by search; the 8
  copies jointly cover all 120 block pairs, multiplicities 1-3). Per core,
  7 lhs groups with contiguous rhs windows of widths (5,5,4,4,3,2,1)
  (24 block-columns total incl. the 7 diagonals) accumulate fp8 DoubleRow
  Gram strips over 48 k-steps of K=256.

  sq (row norms) are extracted on-device from the Gram diagonals
  (identity-mask reduce), transposed via PE, and folded into PSUM with
  bf16 K=1 rank-1 matmuls so PSUM holds p = g - (sq_r + sq_c)/2 = -d2/2.

  Stats use a host-built elementwise weight matrix wmat (1/multiplicity
  per pair block, triangle-mask/diag-mult on diag blocks, 0 on pads):
    E  = sum wmat*exp(2*SS*p),  S1 = sum wmat*p,  S2 = sum wmat*p^2,
    feat = sum wd*sqrt(sq).
  Host combines per-core [128,4] partials in float64.
"""

import os

import numpy as np
import ml_dtypes

import concourse.bass as bass
import concourse.bass_utils as _bu
import concourse.mybir as mybir
import concourse.tile as tile
from concourse import bacc
from concourse.bass_utils import run_bass_kernel_spmd

# (walrus's --enable-ldw-opt=true was tried here: it crashes codegen on
# TRN2 InstLdweights, which is presumably why concourse pins it off)

NC_N = 8
B, D = 2048, 12288
BLK = 128
KCH = 96          # k-chunks of 128
NSTEP = 48        # DoubleRow k-steps of 256
KB = 8            # k-chunks per DMA batch
NBATCH = KCH // KB
TAU = 0.5
SS = 1.0 / (D * TAU)
G = 7             # blocks per core
SLABW = G * BLK   # 896

F32 = mybir.dt.float32
BF16 = mybir.dt.bfloat16
FP16 = mybir.dt.float16
FP8 = mybir.dt.float8e4
NP_FP8 = ml_dtypes.float8_e4m3
NP_BF16 = ml_dtypes.bfloat16

# ---- covering design (found by search; see transcript) ----
W_WIDTHS = (5, 5, 4, 4, 3, 2, 1)
COPIES = [
    [10, 2, 3, 0, 12, 11, 8],
    [14, 8, 11, 10, 1, 9, 15],
    [6, 2, 8, 13, 0, 5, 15],
    [2, 9, 1, 7, 4, 0, 14],
    [8, 7, 15, 4, 3, 11, 6],
    [11, 1, 9, 13, 5, 3, 14],
    [12, 4, 13, 5, 7, 10, 6],
    [2, 15, 1, 6, 14, 12, 9],
]

# PSUM layout: per group (base, [piece col ranges]) bank-padded (512 f32 banks)
GROUP_BASE = (0, 1024, 2048, 2560, 3072, 3584, 3840)
GROUP_PIECES = (
    ((0, 512), (512, 640)),
    ((1024, 1536), (1536, 1664)),
    ((2048, 2560),),
    ((2560, 3072),),
    ((3072, 3456),),
    ((3584, 3840),),
    ((3840, 3968),),
)
PSUM_W = 3968
# transpose scratch inside g0's pad (bank 1, cols 640:768)
PT_OFF = 640
# stats ranges (contiguous real columns)
RANGES = ((0, 640), (1024, 1664), (2048, 3072), (3072, 3456), (3584, 3968))
# engine split: index into RANGES handled by DVE vs GpSimd
DVE_RANGES = (2, 3)
GPS_RANGES = (0, 1, 4)

N_PAIRS = B * (B - 1) // 2

KERNEL_EXEC_NS = []

_cache = {}


def _trace_enabled():
    return bool(os.environ.get("KERNEL_TRACE"))


def _multiplicities():
    from collections import defaultdict

    H = []
    for i, wi in enumerate(W_WIDTHS):
        for j in range(i + 1, i + wi):
            H.append((i, j))
    mult = defaultdict(int)
    dmult = defaultdict(int)
    for phi in COPIES:
        for (i, j) in H:
            a, b = phi[i], phi[j]
            mult[(min(a, b), max(a, b))] += 1
        for b in phi:
            dmult[b] += 1
    return mult, dmult


MULT, DMULT = _multiplicities()


def _build_kernel():
    nc = bacc.Bacc("TRN2", target_bir_lowering=False, debug=False, num_devices=NC_N)
    xT = nc.dram_tensor("xT", [BLK, KCH, SLABW], FP8, kind="ExternalInput")
    wmat_d = nc.dram_tensor("wmat", [BLK, PSUM_W], F32, kind="ExternalInput")
    wmat16_d = nc.dram_tensor("wmat16", [BLK, PSUM_W], FP16, kind="ExternalInput")
    idm_d = nc.dram_tensor("idm", [BLK, BLK], F32, kind="ExternalInput")
    wd_d = nc.dram_tensor("wd", [BLK, 8], F32, kind="ExternalInput")
    out_stats = nc.dram_tensor("out_stats", [BLK, 4], F32, kind="ExternalOutput")

    MULT_OP = mybir.AluOpType.mult
    ADD_OP = mybir.AluOpType.add
    EXP = mybir.ActivationFunctionType.Exp
    SQRT = mybir.ActivationFunctionType.Sqrt
    DR = mybir.MatmulPerfMode.DoubleRow

    with tile.TileContext(nc) as tc:
        with (
            tc.tile_pool(name="slab", bufs=4) as slab_pool,
            tc.tile_pool(name="psp", bufs=1, space="PSUM") as psp,
            tc.tile_pool(name="aux", bufs=1) as aux,
            tc.tile_pool(name="scr", bufs=1) as scr_pool,
            tc.tile_pool(name="accp", bufs=1) as accp,
            tc.tile_pool(name="drp", bufs=1, space="DRAM") as dram_pool,
        ):
            P = psp.tile([BLK, 4096], F32)
            acc = accp.tile([BLK, 16], F32)
            sqcol = accp.tile([BLK, 8], F32)

            # ---- aux inputs + constants ----
            idm = aux.tile([BLK, BLK], F32)
            wdm = aux.tile([BLK, 8], F32)
            onesbf = aux.tile([1, SLABW], BF16)
            nc.gpsimd.memset(onesbf[:], 1.0)

            # PE warm-up + Exp/Sqrt act-table preload while first DMA lands
            warm = aux.tile([BLK, 512], FP8)
            nc.gpsimd.memset(warm[:], 0.0)
            for _ in range(20):
                nc.tensor.matmul(
                    P[:, 3968:4096], warm[:, 0:BLK], warm[:, 0:BLK],
                    start=True, stop=True, skip_group_check=True,
                )
            # pre-zero co-resident regions (start=True resets the whole PSUM
            # bank, so groups sharing a bank must accumulate with start=False
            # onto a zeroed region): g6 (bank 7) and the transpose pad (bank 1)
            nc.tensor.matmul(P[:, 3840:3968], warm[:, 0:BLK], warm[:, 0:BLK],
                             start=True, stop=True, skip_group_check=True)
            nc.tensor.matmul(P[:, 640:768], warm[:, 0:BLK], warm[:, 0:BLK],
                             start=True, stop=True, skip_group_check=True)
            zc = aux.tile([BLK, 1], F32)
            nc.gpsimd.memset(zc[:], 0.0)
            zs = aux.tile([BLK, 1], F32)
            nc.scalar.activation(zs[:], zc[:], SQRT)
            zs2 = aux.tile([BLK, 1], F32)
            nc.scalar.activation(zs2[:], zc[:], EXP)

            # ---- main stream ----
            # kstep 0: bank-aligned pieces with start=True (PSUM bank reset);
            # ksteps 1..47: full-window matmuls (start=False accumulate may
            # cross banks), 7 matmuls per kstep keeps LDWEIGHTS off the
            # critical path. wmat arrives mid-stream (needed only at tail).
            wmat = aux.tile([BLK, PSUM_W], F32)
            wmat16 = aux.tile([BLK, PSUM_W], FP16)
            # first batch split small so the PE can start sooner
            batches = [(0, 2), (2, 6)] + [(k, KB) for k in range(KB, KCH, KB)]
            for bi, (kc0, klen) in enumerate(batches):
                st = slab_pool.tile([BLK, klen, SLABW], FP8, tag=f"slab{min(bi,2)}")
                nc.sync.dma_start(st[:], xT[:, kc0 : kc0 + klen, :])
                if bi == 1:
                    nc.sync.dma_start(idm[:], idm_d[:])
                    nc.sync.dma_start(wdm[:], wd_d[:])
                if bi == 4:
                    nc.sync.dma_start(wmat[:], wmat_d[:])
                    nc.sync.dma_start(wmat16[:], wmat16_d[:])
                for ii in range(0, klen, 2):
                    ks = kc0 + ii  # k-chunk index
                    first = ks == 0
                    last = ks == KCH - 2
                    # bank-aligned pieces (matmul dest must stay in one bank),
                    # ordered big->small to amortize LDWEIGHTS
                    order = [(0, 0), (1, 0), (2, 0), (3, 0), (4, 0),
                             (5, 0), (0, 1), (1, 1), (6, 0)]
                    for (g, pi) in order:
                        pc0, pc1 = GROUP_PIECES[g][pi]
                        base = GROUP_BASE[g]
                        w0 = g * BLK + (pc0 - base)
                        lhs = st[:, ii : ii + 2, g * BLK : (g + 1) * BLK]
                        rhs = st[:, ii : ii + 2, w0 : w0 + (pc1 - pc0)]
                        # g6 shares bank 7 with g5: g5's kstep-0 start resets
                        # the bank; g6 accumulates onto the zeroed region
                        nc.tensor.matmul(
                            P[:, pc0:pc1], lhs, rhs,
                            start=(first and g != 6), stop=last, perf_mode=DR,
                            skip_group_check=(g == 6),
                        )

            # ---- sq extraction from Gram diagonals ----
            # keep-alive zero-accumulate matmuls hold the PE p-state up while
            # DVE extracts and the sq roundtrip completes (adds 0, bank-safe)
            for g in range(G):
                base = GROUP_BASE[g]
                dscr = scr_pool.tile([BLK, BLK], F32, tag=f"dscr{g}")
                nc.vector.scalar_tensor_tensor(
                    out=dscr[:], in0=P[:, base : base + BLK], scalar=1.0,
                    in1=idm[:], op0=MULT_OP, op1=MULT_OP,
                    accum_out=sqcol[:, g : g + 1],
                )
                nc.tensor.matmul(
                    P[:, 768:1024], warm[:, 0:BLK], warm[:, 0:256],
                    start=False, stop=False, skip_group_check=True,
                )
            nc.gpsimd.memset(sqcol[:, 7:8], 0.0)

            # ---- transpose sq to single-partition layout (via PE + DRAM) ----
            pt = P[0:8, PT_OFF : PT_OFF + BLK]
            nc.tensor.matmul(pt, sqcol[:], idm[:], is_transpose=True,
                             start=False, stop=True, skip_group_check=True)
            sqTbf = aux.tile([8, BLK], BF16)
            nc.vector.tensor_scalar(
                out=sqTbf[:], in0=pt, scalar1=-0.5, scalar2=0.0,
                op0=MULT_OP, op1=ADD_OP,
            )
            sqcat = aux.tile([1, 8 * BLK], BF16)
            sq_rt = dram_pool.tile([8, BLK], BF16)
            nc.sync.dma_start(sq_rt[:], sqTbf[:])
            nc.sync.dma_start(
                sqcat[:],
                sq_rt[:].rearrange("p f -> (p f)").rearrange("(a f) -> a f", a=1),
            )
            # N=256 keep-alives (107ns each) into the unread bank-1 pad span
            # the ~4us sq roundtrip so the folds run at full PE clock
            for _ in range(36):
                nc.tensor.matmul(
                    P[:, 768:1024], warm[:, 0:BLK], warm[:, 0:256],
                    start=False, stop=False, skip_group_check=True,
                )

            # ---- rank-1 folds: p := g - sq_r/2 - sq_c/2 (per bank piece) ----
            for g in range(G):
                base = GROUP_BASE[g]
                npieces = len(GROUP_PIECES[g])
                for pi, (pc0, pc1) in enumerate(GROUP_PIECES[g]):
                    lastp = pi == npieces - 1
                    wlen = pc1 - pc0
                    rstart = g * BLK + (pc0 - base)
                    # per-row: adds -sq_lhs[m]/2 to every column
                    nc.tensor.matmul(
                        P[:, pc0:pc1],
                        sqcat[:, g * BLK : (g + 1) * BLK],
                        onesbf[:, 0:wlen],
                        start=False, stop=False, skip_group_check=True,
                    )
                    # per-col: adds -sq_rhs[n]/2 to every row
                    nc.tensor.matmul(
                        P[:, pc0:pc1],
                        onesbf[:, 0:BLK],
                        sqcat[:, rstart : rstart + wlen],
                        start=False, stop=lastp, skip_group_check=True,
                    )

            # ---- stats passes ----
            # acc cols: 0:5 Ew per range, 5:10 S1, 10:15 S2, 15 feat
            # Ew runs on fp16 (DVE 2x); S1/S2 stay f32 from PSUM (fp16/bf16
            # staging of p measurably inflates the pair-std variance).
            # ACT exps emitted first (independent engine); DVE runs all
            # S1/S2 passes (ready as soon as folds land) before the ew ops
            # that depend on ACT output -- the in-order DVE never stalls.
            ets = []
            for ri, (c0, c1) in enumerate(RANGES):
                n = c1 - c0
                et = scr_pool.tile([BLK, n], FP16, tag=f"et{ri}")
                nc.scalar.activation(et[:], P[:, c0:c1], EXP, scale=2.0 * SS)
                ets.append(et)
            for ri, (c0, c1) in enumerate(RANGES):
                n = c1 - c0
                # S1 = sum wmat*p  (f32, out wp reused for S2)
                wp = scr_pool.tile([BLK, n], F32, tag=f"wp{ri}")
                nc.vector.scalar_tensor_tensor(
                    out=wp[:], in0=P[:, c0:c1], scalar=1.0, in1=wmat[:, c0:c1],
                    op0=MULT_OP, op1=MULT_OP, accum_out=acc[:, 5 + ri : 6 + ri],
                )
                # S2 = sum wp*p (f32)
                s2s = scr_pool.tile([BLK, n], F32, tag=f"s2{ri}")
                nc.vector.scalar_tensor_tensor(
                    out=s2s[:], in0=wp[:], scalar=1.0, in1=P[:, c0:c1],
                    op0=MULT_OP, op1=MULT_OP, accum_out=acc[:, 10 + ri : 11 + ri],
                )
            for ri, (c0, c1) in enumerate(RANGES):
                n = c1 - c0
                # E = sum wmat*exp(2*SS*p): DVE 2x fp16 weighting
                ew = scr_pool.tile([BLK, n], FP16, tag=f"ew{ri}")
                nc.vector.scalar_tensor_tensor(
                    out=ew[:], in0=ets[ri][:], scalar=1.0,
                    in1=wmat16[:, c0:c1],
                    op0=MULT_OP, op1=MULT_OP, accum_out=acc[:, ri : 1 + ri],
                )

            # feat partial: sum wd * sqrt(sq)  (single act-table swap, last)
            srt = aux.tile([BLK, 8], F32)
            nc.scalar.activation(srt[:], sqcol[:], SQRT)
            fscr = aux.tile([BLK, 8], F32)
            nc.vector.scalar_tensor_tensor(
                out=fscr[:], in0=srt[:], scalar=1.0, in1=wdm[:],
                op0=MULT_OP, op1=MULT_OP, accum_out=acc[:, 15:16],
            )

            outt = accp.tile([BLK, 4], F32)
            nc.vector.tensor_reduce(outt[:, 0:1], acc[:, 0:5], mybir.AxisListType.X, ADD_OP)
            nc.vector.tensor_reduce(outt[:, 1:2], acc[:, 5:10], mybir.AxisListType.X, ADD_OP)
            nc.vector.tensor_reduce(outt[:, 2:3], acc[:, 10:15], mybir.AxisListType.X, ADD_OP)
            nc.vector.tensor_copy(outt[:, 3:4], acc[:, 15:16])
            nc.sync.dma_start(out_stats[:], outt[:])
    nc.compile()
    return nc


def _get(name, builder):
    if name not in _cache:
        _cache[name] = builder()
    return _cache[name]


def _run(nc, in_maps, tag):
    if _trace_enabled():
        try:
            import profhook

            profhook.install()
        except Exception:
            pass
        import tempfile

        res = run_bass_kernel_spmd(
            nc, in_maps, list(range(NC_N)), trace=True,
            tmpdir=tempfile.mkdtemp(prefix=f"ktrace_{tag}_"),
        )
        KERNEL_EXEC_NS.append((tag, res.exec_time_ns))
        return res.results
    return run_bass_kernel_spmd(nc, in_maps, list(range(NC_N))).results


def _host_inputs(x):
    xq = x.astype(NP_FP8)
    xT = np.ascontiguousarray(xq.T)  # (D, B)
    xTv = xT.reshape(KCH, BLK, B)
    tri = np.triu(np.ones((BLK, BLK), np.float32), k=1)
    idm = np.eye(BLK, dtype=np.float32)
    maps = []
    for c in range(NC_N):
        blocks = COPIES[c]
        slab = np.concatenate(
            [xTv[:, :, b * BLK : (b + 1) * BLK] for b in blocks], axis=2
        ).transpose(1, 0, 2)  # [128, 96, 896]
        wmat = np.zeros((BLK, PSUM_W), np.float32)
        for g in range(G):
            base = GROUP_BASE[g]
            for r in range(g, g + W_WIDTHS[g]):
                col = base + (r - g) * BLK
                if r == g:
                    wmat[:, col : col + BLK] = tri / DMULT[blocks[g]]
                else:
                    a, b = blocks[g], blocks[r]
                    wmat[:, col : col + BLK] = 1.0 / MULT[(min(a, b), max(a, b))]
        wd = np.zeros((BLK, 8), np.float32)
        for g in range(G):
            wd[:, g] = 1.0 / DMULT[blocks[g]]
        maps.append(
            {
                "xT": np.ascontiguousarray(slab),
                "wmat": wmat,
                "wmat16": wmat.astype(np.float16),
                "idm": idm,
                "wd": wd,
            }
        )
    return maps


def kernel(features):
    x = np.asarray(features).reshape(B, D)
    maps = _host_inputs(x)
    nc = _get("main", _build_kernel)
    res = _run(nc, maps, "main")

    E = S1 = S2 = FEAT = 0.0
    for c in range(NC_N):
        o = res[c]["out_stats"].astype(np.float64)
        E += o[:, 0].sum()
        S1 += o[:, 1].sum()
        S2 += o[:, 2].sum()
        FEAT += o[:, 3].sum()

    N = float(N_PAIRS)
    mean_d2 = -2.0 * S1 / N
    mean = mean_d2 / D
    var_d2 = (4.0 * S2 - N * mean_d2 * mean_d2) / (N - 1.0)
    std = np.sqrt(var_d2) / D
    loss = -np.log(E) + np.log(N)
    feat = FEAT / B

    return (
        np.float32(loss),
        np.float32(feat),
        np.float32(mean),
        np.float32(std),
    )


if __name__ == "__main__":
    f = np.random.default_rng(0).standard_normal((B, 16, 768), dtype=np.float32)
    print(kernel(features=f))
